# revision 13
# baseline (speedup 1.0000x reference)
"""Trainium2 Bass kernel for EnhancedLocalComplexAttention.

Reference semantics (complex windowed attention):
  x = x_re + i*x_im                     [b=2, n=4096, dim=512]
  q = x @ wq.T ; k = x @ wk.T ; v = x @ wv.T          (complex, 512x512)
  per head (8 heads x 64) and non-overlapping 128-token window:
    dots = real(q . conj(k)) * scale + rel_bias[j-i+128]
    attn = softmax(dots); out = attn @ v  (attn real)
  y = out @ wo.T  (complex); return stack([y.re, y.im])  [2, b, n, dim]

Sharding: data-parallel over tokens. Core c gets tokens [c*512,(c+1)*512)
of each batch (windows are 128-aligned, so fully local). Weights are
replicated; host pre-transposes them to [k, j] layout, folds the 1/sqrt(d)
scale into wq, and pre-negates the imaginary parts needed for the complex
matmul accumulation, so the device only ever accumulates.

Device layout notes:
  - x is shipped pre-transposed [feat, tok] so the contraction dim sits on
    SBUF partitions for every matmul.
  - q, k are produced transposed [feat, tok] (exactly what the window dots
    need); v and the final output are produced in natural [tok, feat].
  - Projections run as float32r (full PE rate at moving-dim 512);
    attention runs in bf16 with fp32 PSUM/softmax accumulation.
"""

import numpy as np
import ml_dtypes

P = 128         # SBUF partitions / window size
DIM = 512
NKT = DIM // P  # 4 k-tiles
TOK = 512       # tokens per core per batch
NIT = TOK // P  # 4 token tiles (= windows) per chunk
NB = 2          # batches
NH = 8          # heads
HD = 64         # head dim
N_CORES = 8
N = 4096
SCALE = HD ** (-0.5)

W_NAMES = [
    "wq_re_T", "wq_imn_T", "wq_im_T",
    "wk_re_T", "wk_imn_T", "wk_im_T",
    "wv_re_T", "wv_imn_T", "wv_im_T",
    "wo_re_T", "wo_imn_T", "wo_im_T",
]

_COMPILED = {}
LAST_RESULT = None


def _build_program(loop_n=None, phases=("attn", "oproj")):
    import concourse.bacc as bacc
    import concourse.bass as bass
    import concourse.mybir as mybir
    import concourse.tile as tile
    from contextlib import ExitStack

    f32 = mybir.dt.float32
    f32r = mybir.dt.float32r
    bf16 = mybir.dt.bfloat16

    nc = bacc.Bacc(
        "TRN2",
        target_bir_lowering=False,
        debug=False,
        enable_asserts=False,
        num_devices=N_CORES,
    )

    ins = {}
    for name in ["xT_re", "xT_im"]:
        ins[name] = nc.dram_tensor(name, [NB, DIM, TOK], f32r, kind="ExternalInput").ap()
    for name in W_NAMES:
        ins[name] = nc.dram_tensor(name, [DIM, DIM], f32r, kind="ExternalInput").ap()
    ins["bias_mat"] = nc.dram_tensor("bias_mat", [P, P], f32, kind="ExternalInput").ap()
    ins["ident"] = nc.dram_tensor("ident", [P, P], bf16, kind="ExternalInput").ap()
    outs = {
        "y_re": nc.dram_tensor("y_re", [NB, TOK, DIM], f32, kind="ExternalOutput").ap(),
        "y_im": nc.dram_tensor("y_im", [NB, TOK, DIM], f32, kind="ExternalOutput").ap(),
    }

    with tile.TileContext(nc) as tc, ExitStack() as ctx:
        wpool = ctx.enter_context(tc.tile_pool(name="wpool", bufs=1))
        cpool = ctx.enter_context(tc.tile_pool(name="cpool", bufs=1))
        xpool = ctx.enter_context(tc.tile_pool(name="xpool", bufs=2))
        qkpool = ctx.enter_context(tc.tile_pool(name="qkpool", bufs=1))
        vpool = ctx.enter_context(tc.tile_pool(name="vpool", bufs=1))
        aopool = ctx.enter_context(tc.tile_pool(name="aopool", bufs=1))
        ypool = ctx.enter_context(tc.tile_pool(name="ypool", bufs=3))
        sc = ctx.enter_context(tc.tile_pool(name="sc", bufs=3))
        pp_proj = ctx.enter_context(tc.tile_pool(name="pp_proj", bufs=2, space="PSUM"))
        pp_dots = ctx.enter_context(tc.tile_pool(name="pp_dots", bufs=2, space="PSUM"))
        pp_tp = ctx.enter_context(tc.tile_pool(name="pp_tp", bufs=2, space="PSUM"))
        pp_pv = ctx.enter_context(tc.tile_pool(name="pp_pv", bufs=2, space="PSUM"))

        # --- resident constants -------------------------------------------
        w_sb = {}
        w_queues = {"wq": nc.scalar, "wk": nc.gpsimd, "wv": nc.gpsimd, "wo": nc.scalar}
        for name in W_NAMES:
            t = wpool.tile([P, NKT, DIM], f32r, name=f"sb_{name}")
            eng = w_queues[name[:2]]
            eng.dma_start(out=t, in_=ins[name].rearrange("(kt p) j -> p kt j", p=P))
            w_sb[name] = t
        bias_sb = cpool.tile([P, P], f32, name="bias_sb")
        nc.scalar.dma_start(out=bias_sb, in_=ins["bias_mat"])
        id_sb = cpool.tile([P, P], bf16, name="id_sb")
        nc.scalar.dma_start(out=id_sb, in_=ins["ident"])

        def body():
          # x for both batches up-front (xpool bufs=2 holds both)
          x_sb = []
          for b in range(NB):
            xre = xpool.tile([P, NKT, TOK], f32r, tag="xre", name=f"xre{b}")
            nc.sync.dma_start(out=xre, in_=ins["xT_re"][b].rearrange("(kt p) t -> p kt t", p=P))
            xim = xpool.tile([P, NKT, TOK], f32r, tag="xim", name=f"xim{b}")
            nc.sync.dma_start(out=xim, in_=ins["xT_im"][b].rearrange("(kt p) t -> p kt t", p=P))
            x_sb.append((xre, xim))

          for b in range(NB):
            xre, xim = x_sb[b]

            # --- q/k projections -> cat layout [d2(re|im), head, tok] bf16 --
            # partition p<64 holds re of head h (d=p), p>=64 holds im (d=p-64)
            qcat = qkpool.tile([P, NH, TOK], bf16, tag="qcat", name=f"qcat{b}")
            kcat = qkpool.tile([P, NH, TOK], bf16, tag="kcat", name=f"kcat{b}")
            qk_pairs = [
                (qcat, "q", [("wq_re_T", xre), ("wq_imn_T", xim)],
                 [("wq_im_T", xre), ("wq_re_T", xim)]),
                (kcat, "k", [("wk_re_T", xre), ("wk_imn_T", xim)],
                 [("wk_im_T", xre), ("wk_re_T", xim)]),
            ]
            for dst, pnm, re_terms, im_terms in qk_pairs:
                for jt in range(NKT):
                    pss = []
                    for cnm, terms in (("re", re_terms), ("im", im_terms)):
                        ps = pp_proj.tile([P, TOK], f32, tag="proj",
                                          name=f"ps_{pnm}{cnm}{b}{jt}")
                        mm = 0
                        for wname, xs in terms:
                            for kt in range(NKT):
                                nc.tensor.matmul(
                                    ps,
                                    w_sb[wname][:, kt, jt * P:(jt + 1) * P],
                                    xs[:, kt, :],
                                    start=(mm == 0),
                                    stop=(mm == 2 * NKT - 1),
                                )
                                mm += 1
                        pss.append(ps)
                    ps_re, ps_im = pss
                    nc.any.tensor_copy(out=dst[0:HD, 2 * jt, :], in_=ps_re[0:HD, :])
                    nc.any.tensor_copy(out=dst[0:HD, 2 * jt + 1, :], in_=ps_re[HD:P, :])
                    nc.any.tensor_copy(out=dst[HD:P, 2 * jt, :], in_=ps_im[0:HD, :])
                    nc.any.tensor_copy(out=dst[HD:P, 2 * jt + 1, :], in_=ps_im[HD:P, :])

            # --- v projection, natural [tok, it, comp, feat] bf16 ---------
            vcat = vpool.tile([P, NIT, NH, 2, HD], bf16, tag="vcat", name=f"vcat{b}")
            v_terms = [
                [("wv_re_T", xre), ("wv_imn_T", xim)],
                [("wv_im_T", xre), ("wv_re_T", xim)],
            ]
            for comp, terms in enumerate(v_terms):
                for it in range(NIT):
                    ps = pp_proj.tile([P, DIM], f32, tag="proj", name=f"ps_v{comp}{b}{it}")
                    mm = 0
                    for wname, xs in terms:
                        for kt in range(NKT):
                            nc.tensor.matmul(
                                ps,
                                xs[:, kt, it * P:(it + 1) * P],
                                w_sb[wname][:, kt, :],
                                start=(mm == 0),
                                stop=(mm == 2 * NKT - 1),
                            )
                            mm += 1
                    nc.any.tensor_copy(out=vcat[:, it, :, comp, :], in_=ps.rearrange("p (h d) -> p h d", h=NH))

            # --- windowed attention --------------------------------------
            ao = aopool.tile([P, 2, NKT, TOK], f32r, tag="ao", name=f"ao{b}")
            if "attn" not in phases:
                nc.any.tensor_copy(out=ao[:, 0], in_=xre)
                nc.any.tensor_copy(out=ao[:, 1], in_=xim)

            units = [(h, w) for h in range(NH) for w in range(NIT)]
            LA1, LA2 = 1, 5  # software-pipeline offsets
            st_a = {}

            def front(u):
                h, w = units[u]
                wsl = slice(w * P, (w + 1) * P)
                pd = pp_dots.tile([P, P], f32, tag="dots", name=f"pd{b}_{u}")
                nc.tensor.matmul(pd, qcat[:, h, wsl], kcat[:, h, wsl],
                                 start=True, stop=True)
                t = sc.tile([P, P], f32, tag="t", name=f"t{b}_{u}", bufs=3)
                nc.vector.tensor_add(t, pd, bias_sb)
                e = sc.tile([P, P], bf16, tag="e", name=f"e{b}_{u}", bufs=3)
                s = sc.tile([P, 1], f32, tag="s", name=f"s{b}_{u}", bufs=4)
                import concourse.mybir as mybir_
                nc.scalar.activation(out=e, in_=t, func=mybir_.ActivationFunctionType.Exp,
                                     accum_out=s)
                return (e, s)

            def mid(u, es):
                e, s = es
                rcp = sc.tile([P, 1], f32, tag="r", name=f"r{b}_{u}", bufs=4)
                nc.vector.reciprocal(rcp, s)
                a = sc.tile([P, P], bf16, tag="a", name=f"a{b}_{u}", bufs=8)
                nc.vector.tensor_scalar_mul(a, e, rcp)
                return a

            def back(u, a):
                h, w = units[u]
                jt, off = h // 2, (h % 2) * HD
                wsl = slice(w * P, (w + 1) * P)
                pt = pp_tp.tile([P, P], bf16, tag="tp", name=f"pt{b}_{u}")
                nc.tensor.transpose(pt, a, id_sb)
                at = sc.tile([P, P], bf16, tag="at", name=f"at{b}_{u}", bufs=3)
                nc.vector.tensor_copy(out=at, in_=pt)
                pv = pp_pv.tile([P, P], f32, tag="pv", name=f"pv{b}_{u}")
                nc.tensor.matmul(pv, vcat[:, w, h, :, :], at,
                                 start=True, stop=True)
                nc.any.tensor_copy(out=ao[off:off + HD, 0, jt, wsl], in_=pv[0:HD, :])
                nc.any.tensor_copy(out=ao[off:off + HD, 1, jt, wsl], in_=pv[HD:P, :])

            stage_es = {}
            for i in range(len(units) + LA2 if "attn" in phases else 0):
                if i < len(units):
                    stage_es[i] = front(i)
                if LA1 <= i < len(units) + LA1:
                    st_a[i - LA1] = mid(i - LA1, stage_es.pop(i - LA1))
                if i >= LA2:
                    back(i - LA2, st_a.pop(i - LA2))

            # --- output projection, natural [tok, feat] -------------------
            y_terms = {
                "y_re": [(0, "wo_re_T"), (1, "wo_imn_T")],
                "y_im": [(0, "wo_im_T"), (1, "wo_re_T")],
            } if "oproj" in phases else {}
            if "oproj" not in phases:
                live = ypool.tile([P, 32], f32, tag="live", name=f"live{b}")
                srcs = [qcat, kcat, vcat[:, 0, :, 0], ao[:, 0], ao[:, 1]]
                for si, t in enumerate(srcs):
                    nc.any.tensor_copy(out=live[:, si * 4:(si + 1) * 4], in_=t[:, 0, 0:4])
                nc.gpsimd.dma_start(out=outs["y_re"][b, 0:P, 0:32], in_=live)
            for oname, terms in y_terms.items():
                for it in range(NIT):
                    ps = pp_proj.tile([P, DIM], f32, tag="proj", name=f"ps_{oname}{b}{it}")
                    mm = 0
                    for comp, wname in terms:
                        for kt in range(NKT):
                            nc.tensor.matmul(
                                ps,
                                ao[:, comp, kt, it * P:(it + 1) * P],
                                w_sb[wname][:, kt, :],
                                start=(mm == 0),
                                stop=(mm == 2 * NKT - 1),
                            )
                            mm += 1
                    ys = ypool.tile([P, DIM], f32, tag="y", name=f"ys_{oname}{b}{it}")
                    nc.any.tensor_copy(out=ys, in_=ps)
                    nc.gpsimd.dma_start(out=outs[oname][b, it * P:(it + 1) * P, :], in_=ys)

        if loop_n:
            with tc.For_i(0, loop_n):
                body()
        else:
            body()

    nc.compile()
    return nc


def get_compiled(loop_n=None, phases=("attn", "oproj")):
    key = (loop_n, tuple(phases))
    if key not in _COMPILED:
        _COMPILED[key] = _build_program(loop_n, phases)
    return _COMPILED[key]


def make_in_maps(x_re, x_im, wq_re, wq_im, wk_re, wk_im, wv_re, wv_im,
                 wo_re, wo_im, rel_bias):
    """Host-side prep: shard x over token chunks, replicate transposed weights."""
    f32 = np.float32
    x_re = np.asarray(x_re, f32)
    x_im = np.asarray(x_im, f32)

    shared = {
        "wq_re_T": np.ascontiguousarray(np.asarray(wq_re, f32).T * SCALE),
        "wq_im_T": np.ascontiguousarray(np.asarray(wq_im, f32).T * SCALE),
        "wq_imn_T": np.ascontiguousarray(np.asarray(wq_im, f32).T * (-SCALE)),
        "wk_re_T": np.ascontiguousarray(np.asarray(wk_re, f32).T),
        "wk_im_T": np.ascontiguousarray(np.asarray(wk_im, f32).T),
        "wk_imn_T": np.ascontiguousarray(-np.asarray(wk_im, f32).T),
        "wv_re_T": np.ascontiguousarray(np.asarray(wv_re, f32).T),
        "wv_im_T": np.ascontiguousarray(np.asarray(wv_im, f32).T),
        "wv_imn_T": np.ascontiguousarray(-np.asarray(wv_im, f32).T),
        "wo_re_T": np.ascontiguousarray(np.asarray(wo_re, f32).T),
        "wo_im_T": np.ascontiguousarray(np.asarray(wo_im, f32).T),
        "wo_imn_T": np.ascontiguousarray(-np.asarray(wo_im, f32).T),
    }
    idx = np.arange(P)[None, :] - np.arange(P)[:, None] + P
    shared["bias_mat"] = np.ascontiguousarray(np.asarray(rel_bias, f32)[idx])
    shared["ident"] = np.eye(P, dtype=ml_dtypes.bfloat16)

    in_maps = []
    for c in range(N_CORES):
        sl = slice(c * TOK, (c + 1) * TOK)
        m = dict(shared)
        m["xT_re"] = np.ascontiguousarray(x_re[:, sl, :].transpose(0, 2, 1))
        m["xT_im"] = np.ascontiguousarray(x_im[:, sl, :].transpose(0, 2, 1))
        in_maps.append(m)
    return in_maps


def assemble_output(results):
    out = np.empty((2, NB, N, DIM), np.float32)
    for c in range(N_CORES):
        sl = slice(c * TOK, (c + 1) * TOK)
        out[0, :, sl, :] = results[c]["y_re"]
        out[1, :, sl, :] = results[c]["y_im"]
    return out


def kernel(**inputs):
    global LAST_RESULT
    from concourse.bass_utils import run_bass_kernel_spmd

    nc = get_compiled()
    in_maps = make_in_maps(**inputs)
    res = run_bass_kernel_spmd(nc, in_maps, list(range(N_CORES)))
    LAST_RESULT = res
    return assemble_output(res.results)
